# revision 7
# baseline (speedup 1.0000x reference)
"""DGN MLPAgent (2-layer masked graph attention) Trainium2 Bass kernel.

Shapes (hardcoded): B=16384 graphs x N=32 nodes, DIN=275, H=64, A=14.
Data parallel over 8 cores: 2048 graphs (65536 rows) per core.

Device layout strategy (per 512-row iteration = 16 graphs = 4 sub-blocks
of 4 graphs x 32 nodes = 128 partitions):
  - inputs are transposed/padded on host to xT [3, 128, RPC] so the
    encoder contraction dim (din) is on partitions -> no on-chip transpose.
  - h kept transposed [64, 512] (h on partitions, rows on free dim).
  - qkT = relu(WqkT @ hT) one matmul -> [128(q;k), 512].
  - scores per sub-block s: qT_s^T @ kT_s -> [128, 128] block-diagonal
    (4 graphs); cross-graph and masked entries killed by exp(s) * mask
    (mask=0 -> exact 0 since softmax(x*m - 9e15(1-m)) == exp(x)*m/sum).
  - rowsum/recip/scale normalize att; DVE 32x32 stream-transpose gives
    attT exactly (block-diagonal => per-block in-place transpose is the
    full transpose).
  - h2T = v_s^T-free matmul per sub-block; final head + bias via
    tensor_tensor add; [512, 14] contiguous DMA out.
hidden_state is returned unchanged (host pass-through).
"""

import numpy as np

B, N, DIN, H, A = 16384, 32, 275, 64, 14
NCORES = 8
BPC = B // NCORES            # 2048 graphs per core
RPC = BPC * N                # 65536 rows per core
RT = 512                     # rows per device iteration
NIT = RPC // RT              # 128 iterations
NSB = RT // 128              # 4 sub-blocks per iteration
DCH = 3                      # din chunks of 128 (275 -> 384 padded)

_CACHE = {}


def _build_bass():
    import concourse.bass as bass
    import concourse.mybir as mybir
    from concourse import bacc
    from concourse.tile import TileContext

    f32 = mybir.dt.float32
    AF = mybir.ActivationFunctionType
    ALU = mybir.AluOpType

    nc = bacc.Bacc("TRN2", target_bir_lowering=False, debug=False,
                   enable_asserts=False)

    xT = nc.dram_tensor("xT", [DCH, 128, RPC], f32, kind="ExternalInput")
    adjm = nc.dram_tensor("adjm", [NIT, 128, RT], f32, kind="ExternalInput")
    wenc = nc.dram_tensor("wenc", [DCH, 128, H], f32, kind="ExternalInput")
    benc = nc.dram_tensor("benc", [H, 1], f32, kind="ExternalInput")
    wqk1 = nc.dram_tensor("wqk1", [H, 2 * H], f32, kind="ExternalInput")
    wv1 = nc.dram_tensor("wv1", [H, H], f32, kind="ExternalInput")
    wqk2 = nc.dram_tensor("wqk2", [H, 2 * H], f32, kind="ExternalInput")
    wv2 = nc.dram_tensor("wv2", [H, H], f32, kind="ExternalInput")
    wqn = nc.dram_tensor("wqn", [H, A], f32, kind="ExternalInput")
    bqn = nc.dram_tensor("bqn", [128, NSB * A], f32, kind="ExternalInput")
    qout = nc.dram_tensor("qout", [RPC, A], f32, kind="ExternalOutput")


    with TileContext(nc) as tc:
        with (
            tc.tile_pool(name="const", bufs=1) as cpool,
            tc.tile_pool(name="xin", bufs=3) as xpool,
            tc.tile_pool(name="sb", bufs=3) as sb,
            tc.tile_pool(name="ps", bufs=8, space="PSUM") as ps,
        ):
            # --- constants (loaded once) ---
            wenc_t = cpool.tile([128, DCH, H], f32, tag="wenc")
            nc.sync.dma_start(wenc_t[:], wenc[:].rearrange("c p h -> p c h"))
            benc_t = cpool.tile([H, 1], f32, tag="benc")
            nc.sync.dma_start(benc_t[:], benc[:])
            wqk1_t = cpool.tile([H, 2 * H], f32, tag="wqk1")
            nc.sync.dma_start(wqk1_t[:], wqk1[:])
            wv1_t = cpool.tile([H, H], f32, tag="wv1")
            nc.sync.dma_start(wv1_t[:], wv1[:])
            wqk2_t = cpool.tile([H, 2 * H], f32, tag="wqk2")
            nc.sync.dma_start(wqk2_t[:], wqk2[:])
            wv2_t = cpool.tile([H, H], f32, tag="wv2")
            nc.sync.dma_start(wv2_t[:], wv2[:])
            wqn_t = cpool.tile([H, A], f32, tag="wqn")
            nc.sync.dma_start(wqn_t[:], wqn[:])
            bqn_t = cpool.tile([128, NSB * A], f32, tag="bqn")
            nc.sync.dma_start(bqn_t[:], bqn[:])

            wqk = [wqk1_t, wqk2_t]
            wv = [wv1_t, wv2_t]

            for it in range(NIT):
                # block-diagonal mask tile (prebuilt on host)
                mt = sb.tile([128, RT], f32, tag="mask")
                nc.sync.dma_start(mt[:], adjm[it])

                # input tile [128, 3, 512]
                xt = xpool.tile([128, DCH, RT], f32, tag="xt")
                nc.sync.dma_start(
                    xt[:],
                    xT[:, :, it * RT : (it + 1) * RT].rearrange("c p r -> p c r"),
                )

                # encoder: h1T = relu(Wenc^T @ x^T + benc)  [64, 512]
                h_ps = ps.tile([128, RT], f32, tag="ps")
                for c in range(DCH):
                    nc.tensor.matmul(
                        h_ps[:H, :],
                        wenc_t[:, c, :],
                        xt[:, c, :],
                        start=(c == 0),
                        stop=(c == DCH - 1),
                    )
                ht = sb.tile([H, RT], f32, tag="ht")
                nc.scalar.activation(ht[:], h_ps[:H, :], AF.Relu, bias=benc_t[:, 0:1])

                for layer in range(2):
                    # qkT = relu(Wqk^T @ hT)  [128, 512]  (q rows 0-63, k 64-127)
                    qk_ps = ps.tile([128, RT], f32, tag="ps")
                    nc.tensor.matmul(
                        qk_ps[:], wqk[layer][:], ht[:], start=True, stop=True
                    )
                    qkt = sb.tile([128, RT], f32, tag="qkt")
                    nc.scalar.activation(qkt[:], qk_ps[:], AF.Relu)
                    # copy kT half to partition-base-0 tile (scores operands
                    # must share a partition base)
                    ktc = sb.tile([H, RT], f32, tag="ktc")
                    nc.vector.tensor_copy(ktc[:], qkt[H:, :])

                    # v = relu(h @ Wv) in normal layout, per sub-block [128, 64]
                    v_ps = ps.tile([128, NSB * H], f32, tag="ps")
                    for s in range(NSB):
                        nc.tensor.matmul(
                            v_ps[:, s * H : (s + 1) * H],
                            ht[:, s * 128 : (s + 1) * 128],
                            wv[layer][:],
                            start=True,
                            stop=True,
                        )
                    v_sb = sb.tile([128, NSB * H], f32, tag="v")
                    nc.vector.tensor_scalar_max(v_sb[:], v_ps[:], 0.0)

                    # scores per sub-block -> [128, 512] psum
                    sc_ps = ps.tile([128, RT], f32, tag="ps")
                    for s in range(NSB):
                        nc.tensor.matmul(
                            sc_ps[:, s * 128 : (s + 1) * 128],
                            qkt[:H, s * 128 : (s + 1) * 128],
                            ktc[:, s * 128 : (s + 1) * 128],
                            start=True,
                            stop=True,
                        )
                    # att = exp(scores) * mask ; normalize per row
                    e_sb = sb.tile([128, RT], f32, tag="esb")
                    nc.scalar.activation(e_sb[:], sc_ps[:], AF.Exp)
                    am = sb.tile([128, RT], f32, tag="am")
                    nc.vector.tensor_tensor(am[:], e_sb[:], mt[:], ALU.mult)
                    rs = sb.tile([128, NSB], f32, tag="rs")
                    nc.vector.tensor_reduce(
                        rs[:],
                        am[:].rearrange("p (s c) -> p s c", c=128),
                        mybir.AxisListType.X,
                        ALU.add,
                    )
                    rr = sb.tile([128, NSB], f32, tag="rr")
                    nc.vector.reciprocal(rr[:], rs[:])
                    for s in range(NSB):
                        nc.vector.tensor_scalar_mul(
                            am[:, s * 128 : (s + 1) * 128],
                            am[:, s * 128 : (s + 1) * 128],
                            rr[:, s : s + 1],
                        )
                    # attT: block-diag => in-place 32x32 block transpose
                    at = sb.tile([128, RT], f32, tag="at")
                    nc.vector.transpose(at[:], am[:])

                    # h2T[h, n] per sub-block: lhsT = v_s [128m, 64h],
                    # rhs = attT_s [128m, 128n]
                    h2_ps = ps.tile([128, RT], f32, tag="ps")
                    for s in range(NSB):
                        nc.tensor.matmul(
                            h2_ps[:H, s * 128 : (s + 1) * 128],
                            v_sb[:, s * H : (s + 1) * H],
                            at[:, s * 128 : (s + 1) * 128],
                            start=True,
                            stop=True,
                        )
                    ht = sb.tile([H, RT], f32, tag="ht2")
                    nc.vector.tensor_copy(ht[:], h2_ps[:H, :])

                # final: q = h3 @ Wqn + bqn, per sub-block [128, 14]
                fq_ps = ps.tile([128, NSB * A], f32, tag="ps")
                for s in range(NSB):
                    nc.tensor.matmul(
                        fq_ps[:, s * A : (s + 1) * A],
                        ht[:, s * 128 : (s + 1) * 128],
                        wqn_t[:],
                        start=True,
                        stop=True,
                    )
                fq = sb.tile([128, NSB * A], f32, tag="fq")
                nc.vector.tensor_add(fq[:], fq_ps[:], bqn_t[:])
                nc.sync.dma_start(
                    qout[it * RT : (it + 1) * RT, :].rearrange(
                        "(s p) a -> p s a", p=128
                    ),
                    fq[:].rearrange("p (s a) -> p s a", a=A),
                )

    if not nc.is_finalized():
        nc.finalize()
    return nc


def _prep_inputs(inputs, adj, W_enc, b_enc, Wv1, Wk1, Wq1, Wv2, Wk2, Wq2,
                 Wqn, bqn):
    """Host-side shard + layout prep. Returns in_maps for 8 cores."""
    xT_all = np.zeros((NCORES, DCH, 128, RPC), dtype=np.float32)
    inp = np.asarray(inputs, dtype=np.float32).reshape(NCORES, RPC, DIN)
    for c in range(NCORES):
        t = inp[c].T  # [275, RPC] view
        xT_all[c, 0] = t[0:128]
        xT_all[c, 1] = t[128:256]
        xT_all[c, 2, : DIN - 256] = t[256:DIN]

    wenc_pad = np.zeros((DCH, 128, H), dtype=np.float32)
    w = np.asarray(W_enc, dtype=np.float32)
    wenc_pad[0] = w[0:128]
    wenc_pad[1] = w[128:256]
    wenc_pad[2, : DIN - 256] = w[256:DIN]

    common = {
        "wenc": wenc_pad,
        "benc": np.asarray(b_enc, np.float32).reshape(H, 1),
        "wqk1": np.concatenate([Wq1, Wk1], axis=1).astype(np.float32),
        "wv1": np.asarray(Wv1, np.float32),
        "wqk2": np.concatenate([Wq2, Wk2], axis=1).astype(np.float32),
        "wv2": np.asarray(Wv2, np.float32),
        "wqn": np.asarray(Wqn, np.float32),
        "bqn": np.tile(np.asarray(bqn, np.float32), (128, NSB)),
    }
    a6 = np.asarray(adj, np.float32).reshape(NCORES, NIT, NSB, 4, N, N)
    adjc = np.zeros((NCORES, NIT, 4, N, NSB, 4, N), dtype=np.float32)
    for j in range(4):
        adjc[:, :, j, :, :, j, :] = a6[:, :, :, j, :, :].transpose(0, 1, 3, 2, 4)
    adjc = adjc.reshape(NCORES, NIT, 128, RT)
    in_maps = []
    for c in range(NCORES):
        m = dict(common)
        m["xT"] = np.ascontiguousarray(xT_all[c])
        m["adjm"] = adjc[c]
        in_maps.append(m)
    return in_maps


def kernel(inputs, hidden_state, adj, W_enc, b_enc,
           Wv1, bv1, Wk1, bk1, Wq1, bq1,
           Wv2, bv2, Wk2, bk2, Wq2, bq2,
           Wqn, bqn):
    from concourse.bass_utils import run_bass_kernel_spmd

    if "nc" not in _CACHE:
        _CACHE["nc"] = _build_bass()
    nc = _CACHE["nc"]

    in_maps = _prep_inputs(inputs, adj, W_enc, b_enc, Wv1, Wk1, Wq1,
                           Wv2, Wk2, Wq2, Wqn, bqn)
    res = run_bass_kernel_spmd(nc, in_maps, core_ids=list(range(NCORES)))
    q = np.concatenate([r["qout"] for r in res.results], axis=0)
    q = q.reshape(B, N, A)
    return q, np.asarray(hidden_state)
